# revision 41
# baseline (speedup 1.0000x reference)
"""Block-circulant linear layer (CirculantLinear) as a Trainium2 Bass kernel.

Math: the reference computes, per (y, x) grid cell, the circular convolution of
the length-8 eigen vector with the corresponding length-8 input block, summed
over the 128 input blocks (done via FFTs in the reference).  That is exactly a
dense matmul out = x @ W with W[x*8+m, y*8+k] = eigens[y, x, (k-m) % 8], so we
expand the small [128,128,8] eigens parameter into W [1024,1024] on the host
and run a data-parallel dense matmul on 8 NeuronCores (batch sharded, W
replicated).

Layout: each core's batch shard is laid out feature-major ([1024, 4096],
i.e. x^T) when staged for DMA, so the contraction axis lands directly on
SBUF partitions and the PE runs a pure LDWEIGHTS+MATMUL stream — no
on-device transposes.  Device HBM traffic is identical either way.
"""

import os
import sys

import numpy as np

_TRN = "/opt/trn_rl_repo"
if _TRN not in sys.path:
    sys.path.insert(0, _TRN)

# If the image's antenv lacks axon_hooks, stub it so bass_utils' trace
# path (taken when BASS_TRACE=1 is set in the environment) cannot crash.
try:
    import antenv.axon_hooks  # noqa: F401
except Exception:  # pragma: no cover
    import types

    _m = types.ModuleType("antenv.axon_hooks")
    _m._hook = None
    _m.set_axon_ntff_profile_hook = lambda h: setattr(_m, "_hook", h)
    _m.get_axon_ntff_profile_hook = lambda: getattr(_m, "_hook", None)
    sys.modules["antenv.axon_hooks"] = _m

import concourse.bacc as bacc
import concourse.bass as bass
import concourse.mybir as mybir
from concourse.bass_utils import run_bass_kernel_spmd
from concourse.tile import TileContext

_dt = mybir.dt

N_CORES = 8
B, IN_CH, OUT_CH, MINI = 32768, 1024, 1024, 8
GY, GX = OUT_CH // MINI, IN_CH // MINI  # 128, 128
P = 128
BS = B // N_CORES            # rows per core (4096)
KT = IN_CH // P              # contraction tiles (8)
NF = 512                     # matmul moving free dim (one PSUM bank)
NO = OUT_CH // NF            # output halves (2)
SB = 512                     # batch columns per x^T super-tile load
NB = SB // P                 # 128-row output tiles per super-tile (4)

# matmul dtype: float32r streams fp32 at 1 cyc/row (N>=256) vs 4 cyc/row for
# plain float32 (rounded-fp32 / tf32-like precision).  Overridable for A/B.
_MM_DTYPE = {"f32r": _dt.float32r, "f32": _dt.float32}[
    os.environ.get("CIRC_MM_DTYPE", "f32r")
]


def _expand_w(eigens: np.ndarray) -> np.ndarray:
    """eigens [GY, GX, MINI] -> dense W [IN_CH, OUT_CH] of circulant blocks."""
    m = np.arange(MINI)
    k = np.arange(MINI)
    idx = (k[None, :] - m[:, None]) % MINI           # [m, k]
    wb = eigens[:, :, idx]                           # [y, x, m, k]
    w = wb.transpose(1, 2, 0, 3).reshape(IN_CH, OUT_CH)
    return np.ascontiguousarray(w, dtype=np.float32)


def _build_nc(bs: int = BS, mm_dtype=_MM_DTYPE) -> bass.Bass:
    nst = bs // SB           # super-tiles per core
    nc = bacc.Bacc()
    xt_d = nc.declare_dram_parameter("xt", [IN_CH, bs], mm_dtype, isOutput=False)
    w_d = nc.declare_dram_parameter("w", [IN_CH, OUT_CH], mm_dtype, isOutput=False)
    o_d = nc.declare_dram_parameter("out", [bs, OUT_CH], _dt.float32, isOutput=True)

    with TileContext(nc) as tc:
        with (
            tc.tile_pool(name="wpool", bufs=1) as wpool,
            tc.tile_pool(name="xpool", bufs=3) as xpool,
            tc.tile_pool(name="opool", bufs=4) as opool,
            tc.tile_pool(name="pso", bufs=4, space="PSUM") as pso,
        ):
            # Separate tiles per contraction block k — dependency tracking is
            # per-tile, so the k=0 matmuls only wait for the k=0 DMAs instead
            # of the whole 6MB of x-super-tile + W loads.
            def load_xsb(dst_list, s):
                for k in range(KT):
                    nc.sync.dma_start(
                        out=dst_list[k][:],
                        in_=xt_d[k * P : (k + 1) * P, s * SB : (s + 1) * SB],
                    )

            def alloc_xsb(s):
                return [
                    xpool.tile([P, SB], mm_dtype, tag=f"xsb{k}", name=f"xsb{k}_{s}")
                    for k in range(KT)
                ]

            xsbs = {}
            xsbs[0] = alloc_xsb(0)
            # interleave x k=0 / w k=0 first so the first matmul unblocks early
            nc.sync.dma_start(out=xsbs[0][0][:], in_=xt_d[0:P, 0:SB])
            w_tiles = [
                [
                    wpool.tile([P, NF], mm_dtype, tag=f"w{k}_{oh}", name=f"w{k}_{oh}")
                    for oh in range(NO)
                ]
                for k in range(KT)
            ]

            def load_w(k):
                for oh in range(NO):
                    nc.sync.dma_start(
                        out=w_tiles[k][oh][:],
                        in_=w_d[k * P : (k + 1) * P, oh * NF : (oh + 1) * NF],
                    )

            load_w(0)
            for k in range(1, KT):
                nc.sync.dma_start(
                    out=xsbs[0][k][:], in_=xt_d[k * P : (k + 1) * P, 0:SB]
                )
                load_w(k)

            for s in range(nst):
                if s not in xsbs:
                    xsbs[s] = alloc_xsb(s)
                    load_xsb(xsbs[s], s)
                xsb = xsbs[s]
                if s + 1 < nst:
                    # prefetch next super-tile
                    xsbs[s + 1] = alloc_xsb(s + 1)
                    load_xsb(xsbs[s + 1], s + 1)

                for bb in range(NB):
                    b0 = s * SB + bb * P
                    ot = opool.tile([P, OUT_CH], _dt.float32)
                    po = [
                        pso.tile(
                            [P, NF], _dt.float32, tag=f"po{oh}", name=f"po{oh}_{s}_{bb}"
                        )
                        for oh in range(NO)
                    ]
                    # k outer / oh inner: each stationary feeds NO matmuls
                    for k in range(KT):
                        lhs = xsb[k][:, bb * P : (bb + 1) * P]
                        for oh in range(NO):
                            nc.tensor.matmul(
                                po[oh][:],
                                lhsT=lhs,
                                rhs=w_tiles[k][oh][:],
                                start=(k == 0),
                                stop=(k == KT - 1),
                            )
                    # alternate eviction engine so neither DVE nor ACT
                    # rate-limits PSUM recycling; store each half as soon as
                    # its eviction lands so the last store chain is short
                    nc.scalar.copy(ot[:, 0:NF], po[0][:])
                    nc.sync.dma_start(
                        out=o_d[b0 : b0 + P, 0:NF], in_=ot[:, 0:NF]
                    )
                    nc.vector.tensor_copy(ot[:, NF:], po[1][:])
                    nc.sync.dma_start(
                        out=o_d[b0 : b0 + P, NF:], in_=ot[:, NF:]
                    )
    nc.compile()
    return nc


def _run(x: np.ndarray, eigens: np.ndarray, trace: bool = False):
    x = np.ascontiguousarray(x, dtype=np.float32)
    w = _expand_w(np.asarray(eigens, dtype=np.float32))
    nc = _build_nc()
    in_maps = [
        {
            "xt": np.ascontiguousarray(x[i * BS : (i + 1) * BS].T),
            "w": w,
        }
        for i in range(N_CORES)
    ]
    res = run_bass_kernel_spmd(nc, in_maps, list(range(N_CORES)), trace=trace)
    out = np.concatenate(
        [res.results[i]["out"] for i in range(N_CORES)], axis=0
    ).astype(np.float32)
    return out, res


def kernel(x: np.ndarray, eigens: np.ndarray) -> np.ndarray:
    out, _ = _run(x, eigens)
    return out
